# revision 2
# baseline (speedup 1.0000x reference)
"""Trainium2 Bass kernel for nn_ConvLayer: 3x3 conv (stride 1, pad 1) + per-channel offset.

Problem: x[32,64,56,56] (*) w[128,64,3,3] + offset[128,1,1] -> out[32,128,56,56], fp32.

Strategy (8 NeuronCores, data-parallel over batch, 4 images/core):
  - Conv as 9 shifted matmuls (one per 3x3 tap) accumulated in PSUM.
  - CIN=64 -> each tap is a contract-64 matmul = half the 128x128 PE array.
    Two images are processed CONCURRENTLY via 64x128 row tiling: image A's
    channels live in SBUF partitions 0-63 (PE tile (0,0)), image B's in
    partitions 64-127 (PE tile (64,0)). Each accumulates into its own PSUM
    bank; each 64-row tile streams ~1 col/cycle, so the pair reaches full
    PE-array throughput.
  - x and weights are cast to bf16 on the host (PSUM accumulation stays fp32;
    rel err ~5e-3 vs the 2e-2 gate). This halves input HBM traffic, shrinks
    the pipeline-fill head, and enables fast weight loads (FWL).
  - Host pre-pads x to a 58x58 grid (zeros on borders) so every tap is a
    contiguous shifted window; host pre-transposes the weight to [cin,tap,k]
    (lhsT layout) and duplicates it into both partition halves.
  - Weights are loaded tap-0-first (32KB) so the very first matmul is gated
    by ~180KB of DMA, not the full weight blob; the remaining taps stream in
    behind it, always ahead of the matmul that needs them.
  - Output columns are produced on the padded 58-wide grid; the PSUM->SBUF
    eviction (ScalarE for image A, VectorE for image B) compacts back to the
    dense 56-wide grid and fuses the per-channel offset add, so the store DMA
    is fully contiguous.
"""

import numpy as np
from contextlib import ExitStack

import ml_dtypes

import concourse.bass as bass
import concourse.tile as tile
from concourse import bacc, mybir
from concourse.bass_utils import run_bass_kernel_spmd

# Problem constants (hardcoded per contract).
B, CIN, HW, K = 32, 64, 56, 128
NCORES = 8
BPC = B // NCORES          # images per core
HP = HW + 2                # padded row width: 58
NPAD = HP * HP + 4         # padded image + slack for tap reads: 3368
NOUT = HW * HW             # 3136
ROWS_PER_CHUNK = 8
CHUNK = ROWS_PER_CHUNK * HP     # 464 <= 512 (one PSUM bank, fp32)
DCHUNK = ROWS_PER_CHUNK * HW    # 448 dense output cols per chunk
NCHUNKS = HW // ROWS_PER_CHUNK  # 7
TAPS = 9
F32 = mybir.dt.float32
BF16 = mybir.dt.bfloat16

_NC_CACHE = None


def _conv_kernel(ctx: ExitStack, tc: "tile.TileContext", out_ap, xp_ap, w2_ap, off_ap):
    nc = tc.nc
    singles = ctx.enter_context(tc.tile_pool(name="singles", bufs=1))
    xpool = ctx.enter_context(tc.tile_pool(name="xpool", bufs=2))
    opool = ctx.enter_context(tc.tile_pool(name="opool", bufs=2))
    psum = ctx.enter_context(tc.tile_pool(name="psum", bufs=8, space="PSUM"))

    # Chunk groups: first group is a single chunk so its input slice is small
    # and the first matmul starts as early as possible; later groups pair
    # chunks to amortize weight loads. 4 PSUM banks max per group, 8 total
    # with double buffering.
    groups = [(0,), (1, 2), (3, 4), (5,), (6,)]
    # x-load slices: slice g covers every tap read of chunk group g
    # (chunk c reads cols < 464*c + 582), so group g's matmuls gate only on
    # slices <= g.
    xbounds = [0, 584, 1512, 2440, 2904, NPAD]
    # Output store slices: one per chunk group (dense cols).
    obounds = [0, 1 * DCHUNK, 3 * DCHUNK, 5 * DCHUNK, 6 * DCHUNK, NOUT]

    # Weights as lhsT [c, tap, k], duplicated across both partition halves.
    # Tap 0 is dispatched first (32KB) so the first matmul's gate is tiny;
    # the rest follows split across both HWDGE rings, racing ahead of the
    # tap-1..8 matmuls of chunk 0.
    w_sb = singles.tile([128, TAPS, K], BF16)
    nc.sync.dma_start(w_sb[:, 0], w2_ap[:, 0])
    nc.scalar.dma_start(w_sb[:, 1:5], w2_ap[:, 1:5])
    nc.sync.dma_start(w_sb[:, 5:9], w2_ap[:, 5:9])
    off_sb = singles.tile([128, 1], F32)
    nc.scalar.dma_start(off_sb[:], off_ap[:])

    # PE warmup: cheap bf16 matmuls on scratch keep TensorE busy during the
    # input-DMA head so the HAM clock gate opens (1.2 -> 2.4 GHz) early.
    # Few enough that they finish before the first input slice lands (the PE
    # queue is FIFO, so excess warmups would delay the real matmuls).
    scratch = singles.tile([128, 512], BF16)
    nc.vector.memset(scratch[:], 0.0)
    ps_warm = psum.tile([128, 512], F32, tag="ps", name="ps_warm")
    for _ in range(4):
        nc.tensor.matmul(
            ps_warm[:], lhsT=scratch[0:64, 0:128], rhs=scratch[0:64, :],
            start=True, stop=True,
        )

    for pair in range(BPC // 2):
        b0 = 2 * pair
        # Both images of the pair side by side: [2, CIN, NPAD] -> [128, NPAD],
        # split into 5 column slices so early chunk groups start ASAP.
        x_t = xpool.tile([128, NPAD], BF16, tag="x")
        xsrc = xp_ap[b0 : b0 + 2].rearrange("b c n -> (b c) n")
        for s in range(len(xbounds) - 1):
            # Alternate the two HWDGE rings so input slices drain in parallel.
            eng = nc.scalar if s % 2 == 0 else nc.sync
            eng.dma_start(
                x_t[:, xbounds[s] : xbounds[s + 1]],
                xsrc[:, xbounds[s] : xbounds[s + 1]],
            )
        o_sb = [
            opool.tile([128, NOUT], F32, tag="oA", name=f"oA_{pair}"),
            opool.tile([128, NOUT], F32, tag="oB", name=f"oB_{pair}"),
        ]

        for g, grp in enumerate(groups):
            ps = {}
            for half in (0, 1):
                for c in grp:
                    ps[(half, c)] = psum.tile(
                        [128, CHUNK], F32, tag="ps", name=f"ps_{pair}_{half}_{c}"
                    )
            for t in range(TAPS):
                kh, kw = divmod(t, 3)
                o = kh * HP + kw
                st, sp = (t == 0), (t == TAPS - 1)
                for half in (0, 1):
                    lo, hi = 64 * half, 64 * half + 64
                    for c in grp:
                        nc.tensor.matmul(
                            ps[(half, c)][:],
                            lhsT=w_sb[lo:hi, t, :],
                            rhs=x_t[lo:hi, o + CHUNK * c : o + CHUNK * c + CHUNK],
                            start=st,
                            stop=sp,
                        )
            # Evict: compact 58-wide padded rows to 56-wide dense rows and add
            # the per-channel offset. Image A on ScalarE, image B on VectorE.
            for c in grp:
                pa = ps[(0, c)].rearrange("p (r x) -> p r x", x=HP)[:, :, 0:HW]
                oa = o_sb[0][:, c * DCHUNK : (c + 1) * DCHUNK].rearrange(
                    "p (r x) -> p r x", x=HW
                )
                nc.scalar.add(oa, pa, off_sb)
                pb = ps[(1, c)].rearrange("p (r x) -> p r x", x=HP)[:, :, 0:HW]
                ob = o_sb[1][:, c * DCHUNK : (c + 1) * DCHUNK].rearrange(
                    "p (r x) -> p r x", x=HW
                )
                nc.vector.tensor_scalar_add(ob, pb, off_sb)
            # Stream this group's output slice out immediately. Image A rides
            # the Scalar HWDGE ring, image B the Sync ring, so the two output
            # streams (and the input stream) drain in parallel.
            nc.scalar.dma_start(
                out_ap[b0][:, obounds[g] : obounds[g + 1]],
                o_sb[0][:, obounds[g] : obounds[g + 1]],
            )
            nc.sync.dma_start(
                out_ap[b0 + 1][:, obounds[g] : obounds[g + 1]],
                o_sb[1][:, obounds[g] : obounds[g + 1]],
            )


def _build_nc():
    global _NC_CACHE
    if _NC_CACHE is not None:
        return _NC_CACHE
    nc = bacc.Bacc(
        "TRN2", target_bir_lowering=False, debug=False, num_devices=NCORES
    )
    xp_ap = nc.dram_tensor("xp", [BPC, CIN, NPAD], BF16, kind="ExternalInput").ap()
    w2_ap = nc.dram_tensor("w2", [128, TAPS, K], BF16, kind="ExternalInput").ap()
    off_ap = nc.dram_tensor("off", [K, 1], F32, kind="ExternalInput").ap()
    out_ap = nc.dram_tensor("out", [BPC, K, NOUT], F32, kind="ExternalOutput").ap()
    with tile.TileContext(nc) as tc:
        with ExitStack() as ctx:
            _conv_kernel(ctx, tc, out_ap, xp_ap, w2_ap, off_ap)
    nc.compile()
    _NC_CACHE = nc
    return nc


def _prep_inputs(x, weight, offset):
    """Host-side layout prep: pad x, transpose+duplicate weights, cast bf16."""
    x = np.ascontiguousarray(np.asarray(x, dtype=np.float32))
    weight = np.asarray(weight, dtype=np.float32)
    offset = np.asarray(offset, dtype=np.float32)

    xph = np.zeros((B, CIN, NPAD), dtype=ml_dtypes.bfloat16)
    xph[:, :, : HP * HP].reshape(B, CIN, HP, HP)[:, :, 1 : 1 + HW, 1 : 1 + HW] = x
    xph = np.ascontiguousarray(xph)

    wt = np.ascontiguousarray(weight.transpose(1, 2, 3, 0)).reshape(CIN, TAPS, K)
    w2 = np.ascontiguousarray(
        np.concatenate([wt, wt], axis=0).astype(ml_dtypes.bfloat16)
    )  # [128, 9, 128]
    off = np.ascontiguousarray(offset.reshape(K, 1))
    return xph, w2, off


def kernel(x, weight, offset):
    nc = _build_nc()
    xph, w2, off = _prep_inputs(x, weight, offset)
    in_maps = [
        {"xp": xph[i * BPC : (i + 1) * BPC], "w2": w2, "off": off}
        for i in range(NCORES)
    ]
    res = run_bass_kernel_spmd(nc, in_maps, list(range(NCORES))).results
    out = np.concatenate(
        [res[i]["out"].reshape(BPC, K, HW, HW) for i in range(NCORES)], axis=0
    )
    return out


# revision 3
# speedup vs baseline: 1.0438x; 1.0438x over previous
"""Trainium2 Bass kernel for nn_ConvLayer: 3x3 conv (stride 1, pad 1) + per-channel offset.

Problem: x[32,64,56,56] (*) w[128,64,3,3] + offset[128,1,1] -> out[32,128,56,56], fp32.

Strategy (8 NeuronCores, data-parallel over batch, 4 images/core):
  - Conv as 9 shifted matmuls (one per 3x3 tap) accumulated in PSUM.
  - CIN=64 -> each tap is a contract-64 matmul = half the 128x128 PE array.
    Two images are processed CONCURRENTLY via 64x128 row tiling: image A's
    channels live in SBUF partitions 0-63 (PE tile (0,0)), image B's in
    partitions 64-127 (PE tile (64,0)). Each accumulates into its own PSUM
    bank; each 64-row tile streams ~1 col/cycle, so the pair reaches full
    PE-array throughput.
  - x and weights are cast to bf16 on the host (PSUM accumulation stays fp32;
    rel err ~5e-3 vs the 2e-2 gate). This halves input HBM traffic, shrinks
    the pipeline-fill head, and enables fast weight loads (FWL).
  - Host pre-pads x to a 58x58 grid (zeros on borders) so every tap is a
    contiguous shifted window; host pre-transposes the weight to [cin,tap,k]
    (lhsT layout) and duplicates it into both partition halves.
  - Weights are loaded tap-0-first (32KB) so the very first matmul is gated
    by ~180KB of DMA, not the full weight blob; the remaining taps stream in
    behind it, always ahead of the matmul that needs them.
  - Output columns are produced on the padded 58-wide grid; the PSUM->SBUF
    eviction (ScalarE for image A, VectorE for image B) compacts back to the
    dense 56-wide grid and fuses the per-channel offset add, so the store DMA
    is fully contiguous.
"""

import numpy as np
from contextlib import ExitStack

import ml_dtypes

import concourse.bass as bass
import concourse.tile as tile
from concourse import bacc, mybir
from concourse.bass_utils import run_bass_kernel_spmd

# Problem constants (hardcoded per contract).
B, CIN, HW, K = 32, 64, 56, 128
NCORES = 8
BPC = B // NCORES          # images per core
HP = HW + 2                # padded row width: 58
NPAD = HP * HP + 4         # padded image + slack for tap reads: 3368
NOUT = HW * HW             # 3136
ROWS_PER_CHUNK = 8
CHUNK = ROWS_PER_CHUNK * HP     # 464 <= 512 (one PSUM bank, fp32)
DCHUNK = ROWS_PER_CHUNK * HW    # 448 dense output cols per chunk
NCHUNKS = HW // ROWS_PER_CHUNK  # 7
TAPS = 9
F32 = mybir.dt.float32
BF16 = mybir.dt.bfloat16

_NC_CACHE = None


def _conv_kernel(ctx: ExitStack, tc: "tile.TileContext", out_ap, xp_ap, w2_ap, off_ap):
    nc = tc.nc
    singles = ctx.enter_context(tc.tile_pool(name="singles", bufs=1))
    xpool = ctx.enter_context(tc.tile_pool(name="xpool", bufs=2))
    opool = ctx.enter_context(tc.tile_pool(name="opool", bufs=2))
    psum = ctx.enter_context(tc.tile_pool(name="psum", bufs=8, space="PSUM"))

    # Chunk groups: first group is a single chunk so its input slice is small
    # and the first matmul starts as early as possible; later groups pair
    # chunks to amortize weight loads. 4 PSUM banks max per group, 8 total
    # with double buffering.
    groups = [(0,), (1, 2), (3, 4), (5,), (6,)]
    # x-load slices: slice g covers every tap read of chunk group g
    # (chunk c reads cols < 464*c + 582), so group g's matmuls gate only on
    # slices <= g.
    xbounds = [0, 584, 1512, 2440, 2904, NPAD]
    # Output store slices: one per chunk group (dense cols).
    obounds = [0, 1 * DCHUNK, 3 * DCHUNK, 5 * DCHUNK, 6 * DCHUNK, NOUT]

    # Memset the warmup scratch first: it is SBUF-local and gates the first
    # warmup matmul, so it must not sit behind any DMA in program order.
    scratch = singles.tile([128, 512], BF16)
    nc.vector.memset(scratch[:], 0.0)

    # Head DMA order is critical: the first real matmul is gated by tap-0
    # weights + x slice 0, so those two ride the front of their rings. The
    # remaining weight taps are split so each tap's DMA completes just ahead
    # of the matmul that consumes it, racing the ~200ns/tap matmul cadence.
    #   Sync ring:   w tap0 (32KB) -> w taps1-4 (131KB) -> x slice1 ...
    #   Scalar ring: x slice0 (150KB) -> w taps5-8 (131KB) -> off -> x slice2
    w_sb = singles.tile([128, TAPS, K], BF16)
    off_sb = singles.tile([128, 1], F32)
    x_ts = []
    x_srcs = []
    for pair in range(BPC // 2):
        x_ts.append(xpool.tile([128, NPAD], BF16, tag="x", name=f"x_{pair}"))
        x_srcs.append(xp_ap[2 * pair : 2 * pair + 2].rearrange("b c n -> (b c) n"))

    def load_x_slice(pair, s, eng):
        eng.dma_start(
            x_ts[pair][:, xbounds[s] : xbounds[s + 1]],
            x_srcs[pair][:, xbounds[s] : xbounds[s + 1]],
        )

    nc.sync.dma_start(w_sb[:, 0], w2_ap[:, 0])
    load_x_slice(0, 0, nc.scalar)
    nc.sync.dma_start(w_sb[:, 1:5], w2_ap[:, 1:5])
    nc.scalar.dma_start(w_sb[:, 5:9], w2_ap[:, 5:9])
    nc.scalar.dma_start(off_sb[:], off_ap[:])

    # PE warmup: cheap bf16 matmuls on scratch keep TensorE busy during the
    # input-DMA head so the HAM clock gate opens (1.2 -> 2.4 GHz) early.
    # Few enough that they finish about when the first input slice lands (the
    # PE queue is FIFO, so excess warmups would delay the real matmuls).
    ps_warm = psum.tile([128, 512], F32, tag="ps", name="ps_warm")
    for _ in range(4):
        nc.tensor.matmul(
            ps_warm[:], lhsT=scratch[0:64, 0:128], rhs=scratch[0:64, :],
            start=True, stop=True,
        )

    for pair in range(BPC // 2):
        b0 = 2 * pair
        # Both images of the pair side by side: [2, CIN, NPAD] -> [128, NPAD],
        # split into 5 column slices so early chunk groups start ASAP.
        x_t = x_ts[pair]
        for s in range(len(xbounds) - 1):
            if pair == 0 and s == 0:
                continue  # already dispatched at the head
            # Alternate the two HWDGE rings so input slices drain in parallel.
            eng = nc.sync if s % 2 == 1 else nc.scalar
            load_x_slice(pair, s, eng)
        o_sb = [
            opool.tile([128, NOUT], F32, tag="oA", name=f"oA_{pair}"),
            opool.tile([128, NOUT], F32, tag="oB", name=f"oB_{pair}"),
        ]

        for g, grp in enumerate(groups):
            ps = {}
            for half in (0, 1):
                for c in grp:
                    ps[(half, c)] = psum.tile(
                        [128, CHUNK], F32, tag="ps", name=f"ps_{pair}_{half}_{c}"
                    )
            for t in range(TAPS):
                kh, kw = divmod(t, 3)
                o = kh * HP + kw
                st, sp = (t == 0), (t == TAPS - 1)
                for half in (0, 1):
                    lo, hi = 64 * half, 64 * half + 64
                    for c in grp:
                        nc.tensor.matmul(
                            ps[(half, c)][:],
                            lhsT=w_sb[lo:hi, t, :],
                            rhs=x_t[lo:hi, o + CHUNK * c : o + CHUNK * c + CHUNK],
                            start=st,
                            stop=sp,
                        )
            # Evict: compact 58-wide padded rows to 56-wide dense rows and add
            # the per-channel offset. Image A on ScalarE, image B on VectorE.
            for c in grp:
                pa = ps[(0, c)].rearrange("p (r x) -> p r x", x=HP)[:, :, 0:HW]
                oa = o_sb[0][:, c * DCHUNK : (c + 1) * DCHUNK].rearrange(
                    "p (r x) -> p r x", x=HW
                )
                nc.scalar.add(oa, pa, off_sb)
                pb = ps[(1, c)].rearrange("p (r x) -> p r x", x=HP)[:, :, 0:HW]
                ob = o_sb[1][:, c * DCHUNK : (c + 1) * DCHUNK].rearrange(
                    "p (r x) -> p r x", x=HW
                )
                nc.vector.tensor_scalar_add(ob, pb, off_sb)
            # Stream this group's output slice out immediately. Image A rides
            # the Scalar HWDGE ring, image B the Sync ring, so the two output
            # streams (and the input stream) drain in parallel.
            nc.scalar.dma_start(
                out_ap[b0][:, obounds[g] : obounds[g + 1]],
                o_sb[0][:, obounds[g] : obounds[g + 1]],
            )
            nc.sync.dma_start(
                out_ap[b0 + 1][:, obounds[g] : obounds[g + 1]],
                o_sb[1][:, obounds[g] : obounds[g + 1]],
            )


def _build_nc():
    global _NC_CACHE
    if _NC_CACHE is not None:
        return _NC_CACHE
    nc = bacc.Bacc(
        "TRN2", target_bir_lowering=False, debug=False, num_devices=NCORES
    )
    xp_ap = nc.dram_tensor("xp", [BPC, CIN, NPAD], BF16, kind="ExternalInput").ap()
    w2_ap = nc.dram_tensor("w2", [128, TAPS, K], BF16, kind="ExternalInput").ap()
    off_ap = nc.dram_tensor("off", [K, 1], F32, kind="ExternalInput").ap()
    out_ap = nc.dram_tensor("out", [BPC, K, NOUT], F32, kind="ExternalOutput").ap()
    with tile.TileContext(nc) as tc:
        with ExitStack() as ctx:
            _conv_kernel(ctx, tc, out_ap, xp_ap, w2_ap, off_ap)
    nc.compile()
    _NC_CACHE = nc
    return nc


def _prep_inputs(x, weight, offset):
    """Host-side layout prep: pad x, transpose+duplicate weights, cast bf16."""
    x = np.ascontiguousarray(np.asarray(x, dtype=np.float32))
    weight = np.asarray(weight, dtype=np.float32)
    offset = np.asarray(offset, dtype=np.float32)

    xph = np.zeros((B, CIN, NPAD), dtype=ml_dtypes.bfloat16)
    xph[:, :, : HP * HP].reshape(B, CIN, HP, HP)[:, :, 1 : 1 + HW, 1 : 1 + HW] = x
    xph = np.ascontiguousarray(xph)

    wt = np.ascontiguousarray(weight.transpose(1, 2, 3, 0)).reshape(CIN, TAPS, K)
    w2 = np.ascontiguousarray(
        np.concatenate([wt, wt], axis=0).astype(ml_dtypes.bfloat16)
    )  # [128, 9, 128]
    off = np.ascontiguousarray(offset.reshape(K, 1))
    return xph, w2, off


def kernel(x, weight, offset):
    nc = _build_nc()
    xph, w2, off = _prep_inputs(x, weight, offset)
    in_maps = [
        {"xp": xph[i * BPC : (i + 1) * BPC], "w2": w2, "off": off}
        for i in range(NCORES)
    ]
    res = run_bass_kernel_spmd(nc, in_maps, list(range(NCORES))).results
    out = np.concatenate(
        [res[i]["out"].reshape(BPC, K, HW, HW) for i in range(NCORES)], axis=0
    )
    return out


# revision 6
# speedup vs baseline: 1.0846x; 1.0391x over previous
"""Trainium2 Bass kernel for nn_ConvLayer: 3x3 conv (stride 1, pad 1) + per-channel offset.

Problem: x[32,64,56,56] (*) w[128,64,3,3] + offset[128,1,1] -> out[32,128,56,56], fp32.

Strategy (8 NeuronCores, data-parallel over batch, 4 images/core):
  - Conv as 9 shifted matmuls (one per 3x3 tap) accumulated in PSUM.
  - CIN=64 -> each tap is a contract-64 matmul = half the 128x128 PE array.
    Two images are processed CONCURRENTLY via 64x128 row tiling: image A's
    channels live in SBUF partitions 0-63 (PE tile (0,0)), image B's in
    partitions 64-127 (PE tile (64,0)). Each accumulates into its own PSUM
    bank; each 64-row tile streams ~1 col/cycle, so the pair reaches full
    PE-array throughput.
  - x and weights are cast to bf16 on the host (PSUM accumulation stays fp32;
    rel err ~5e-3 vs the 2e-2 gate). This halves input HBM traffic, shrinks
    the pipeline-fill head, and enables fast weight loads (FWL).
  - Host pre-pads x to a 58x58 grid (zeros on borders) so every tap is a
    contiguous shifted window; host pre-transposes the weight to [cin,tap,k]
    (lhsT layout) and duplicates it into both partition halves.
  - Weights are loaded tap-0-first (32KB) so the very first matmul is gated
    by ~180KB of DMA, not the full weight blob; the remaining taps stream in
    behind it, always ahead of the matmul that needs them.
  - Output columns are produced on the padded 58-wide grid; the PSUM->SBUF
    eviction (ScalarE for image A, VectorE for image B) compacts back to the
    dense 56-wide grid and fuses the per-channel offset add, so the store DMA
    is fully contiguous.
"""

import numpy as np
from contextlib import ExitStack

import ml_dtypes

import concourse.bass as bass
import concourse.tile as tile
from concourse import bacc, mybir
from concourse.bass_utils import run_bass_kernel_spmd

# Problem constants (hardcoded per contract).
B, CIN, HW, K = 32, 64, 56, 128
NCORES = 8
BPC = B // NCORES          # images per core
HP = HW + 2                # padded row width: 58
NPAD = HP * HP + 4         # padded image + slack for tap reads: 3368
NOUT = HW * HW             # 3136
ROWS_PER_CHUNK = 8
CHUNK = ROWS_PER_CHUNK * HP     # 464 <= 512 (one PSUM bank, fp32)
DCHUNK = ROWS_PER_CHUNK * HW    # 448 dense output cols per chunk
NCHUNKS = HW // ROWS_PER_CHUNK  # 7
TAPS = 9
F32 = mybir.dt.float32
BF16 = mybir.dt.bfloat16

_NC_CACHE = None


def _conv_kernel(ctx: ExitStack, tc: "tile.TileContext", out_ap, xp_ap, w2_ap, off_ap):
    nc = tc.nc
    singles = ctx.enter_context(tc.tile_pool(name="singles", bufs=1))
    xpool = ctx.enter_context(tc.tile_pool(name="xpool", bufs=2))
    opool = ctx.enter_context(tc.tile_pool(name="opool", bufs=2))
    psum = ctx.enter_context(tc.tile_pool(name="psum", bufs=8, space="PSUM"))

    # Chunk groups: first group is a single chunk so its input slice is small
    # and the first matmul starts as early as possible; later groups pair
    # chunks to amortize weight loads. 4 PSUM banks max per group, 8 total
    # with double buffering.
    groups = [(0,), (1, 2), (3, 4), (5,), (6,)]
    # x-load slices: slice g covers every tap read of chunk group g
    # (chunk c reads cols < 464*c + 582), so group g's matmuls gate only on
    # slices <= g.
    xbounds = [0, 584, 1512, 2440, 2904, NPAD]
    # Output store slices: one per chunk group (dense cols).
    obounds = [0, 1 * DCHUNK, 3 * DCHUNK, 5 * DCHUNK, 6 * DCHUNK, NOUT]

    # Memset the warmup scratch first: it is SBUF-local and gates the first
    # warmup matmul, so it must not sit behind any DMA in program order.
    scratch = singles.tile([128, 512], BF16)
    nc.vector.memset(scratch[:], 0.0)

    # Head DMA order is critical: the first real matmul is gated by tap-0
    # weights + x slice 0, so those two ride the front of their rings. The
    # remaining weight taps are split so each tap's DMA completes just ahead
    # of the matmul that consumes it, racing the ~200ns/tap matmul cadence.
    #   Sync ring:   w tap0 (32KB) -> w taps1-4 (131KB) -> x slice1 ...
    #   Scalar ring: x slice0 (150KB) -> w taps5-8 (131KB) -> off -> x slice2
    w_sb = singles.tile([128, TAPS, K], BF16)
    off_sb = singles.tile([128, 1], F32)
    x_ts = []
    x_srcs = []
    for pair in range(BPC // 2):
        x_ts.append(xpool.tile([128, NPAD], BF16, tag="x", name=f"x_{pair}"))
        x_srcs.append(xp_ap[2 * pair : 2 * pair + 2].rearrange("b c n -> (b c) n"))

    def load_x_slice(pair, s, eng):
        eng.dma_start(
            x_ts[pair][:, xbounds[s] : xbounds[s + 1]],
            x_srcs[pair][:, xbounds[s] : xbounds[s + 1]],
        )

    nc.sync.dma_start(w_sb[:, 0], w2_ap[:, 0])
    load_x_slice(0, 0, nc.scalar)
    nc.sync.dma_start(w_sb[:, 1:5], w2_ap[:, 1:5])
    nc.scalar.dma_start(w_sb[:, 5:9], w2_ap[:, 5:9])
    nc.scalar.dma_start(off_sb[:], off_ap[:])

    # PE warmup: cheap bf16 matmuls on scratch keep TensorE busy during the
    # input-DMA head so the HAM clock gate opens (1.2 -> 2.4 GHz) early. The
    # HAM busy-window resets on ANY idle gap, so the warmups must bridge all
    # the way to the (jittery, ~9.4-11.4us) first-input-landing time; sizes
    # decrease so the tail is fine-grained and real matmuls slot in with at
    # most ~128 cycles of queue delay.
    ps_warm = psum.tile([128, 512], F32, tag="ps", name="ps_warm")
    for wn in (512, 512, 512, 512, 512, 256, 256, 256, 128, 128, 128, 128):
        nc.tensor.matmul(
            ps_warm[:, :wn], lhsT=scratch[0:64, 0:128], rhs=scratch[0:64, :wn],
            start=True, stop=True,
        )

    for pair in range(BPC // 2):
        b0 = 2 * pair
        # Both images of the pair side by side: [2, CIN, NPAD] -> [128, NPAD],
        # split into 5 column slices so early chunk groups start ASAP.
        x_t = x_ts[pair]
        for s in range(len(xbounds) - 1):
            if pair == 0 and s == 0:
                continue  # already dispatched at the head
            # Alternate the two HWDGE rings so input slices drain in parallel.
            eng = nc.sync if s % 2 == 1 else nc.scalar
            load_x_slice(pair, s, eng)
        o_sb = [
            opool.tile([128, NOUT], F32, tag="oA", name=f"oA_{pair}"),
            opool.tile([128, NOUT], F32, tag="oB", name=f"oB_{pair}"),
        ]

        for g, grp in enumerate(groups):
            ps = {}
            for half in (0, 1):
                for c in grp:
                    ps[(half, c)] = psum.tile(
                        [128, CHUNK], F32, tag="ps", name=f"ps_{pair}_{half}_{c}"
                    )
            for t in range(TAPS):
                kh, kw = divmod(t, 3)
                o = kh * HP + kw
                st, sp = (t == 0), (t == TAPS - 1)
                for half in (0, 1):
                    lo, hi = 64 * half, 64 * half + 64
                    for c in grp:
                        nc.tensor.matmul(
                            ps[(half, c)][:],
                            lhsT=w_sb[lo:hi, t, :],
                            rhs=x_t[lo:hi, o + CHUNK * c : o + CHUNK * c + CHUNK],
                            start=st,
                            stop=sp,
                        )
            # Evict: compact 58-wide padded rows to 56-wide dense rows and add
            # the per-channel offset. Image A on ScalarE, image B on VectorE.
            # The very last group of the last pair is evicted and stored in
            # row-halves so the final (critical-path) store completion covers
            # half the data and starts ~0.35us earlier.
            last_tail = pair == BPC // 2 - 1 and g == len(groups) - 1
            halves = ((0, 4), (4, 8)) if last_tail else ((0, 8),)
            for c in grp:
                for r0, r1 in halves:
                    pa = ps[(0, c)].rearrange("p (r x) -> p r x", x=HP)[:, r0:r1, 0:HW]
                    oa = o_sb[0][:, c * DCHUNK : (c + 1) * DCHUNK].rearrange(
                        "p (r x) -> p r x", x=HW
                    )[:, r0:r1]
                    nc.scalar.add(oa, pa, off_sb)
                    pb = ps[(1, c)].rearrange("p (r x) -> p r x", x=HP)[:, r0:r1, 0:HW]
                    ob = o_sb[1][:, c * DCHUNK : (c + 1) * DCHUNK].rearrange(
                        "p (r x) -> p r x", x=HW
                    )[:, r0:r1]
                    nc.vector.tensor_scalar_add(ob, pb, off_sb)
                    if last_tail:
                        lo = c * DCHUNK + r0 * HW
                        hi = c * DCHUNK + r1 * HW
                        nc.scalar.dma_start(out_ap[b0][:, lo:hi], o_sb[0][:, lo:hi])
                        nc.sync.dma_start(out_ap[b0 + 1][:, lo:hi], o_sb[1][:, lo:hi])
            if not last_tail:
                # Stream this group's output slice out immediately. Image A
                # rides the Scalar HWDGE ring, image B the Sync ring, so the
                # two output streams (and the input stream) drain in parallel.
                nc.scalar.dma_start(
                    out_ap[b0][:, obounds[g] : obounds[g + 1]],
                    o_sb[0][:, obounds[g] : obounds[g + 1]],
                )
                nc.sync.dma_start(
                    out_ap[b0 + 1][:, obounds[g] : obounds[g + 1]],
                    o_sb[1][:, obounds[g] : obounds[g + 1]],
                )


def _build_nc():
    global _NC_CACHE
    if _NC_CACHE is not None:
        return _NC_CACHE
    nc = bacc.Bacc(
        "TRN2", target_bir_lowering=False, debug=False, num_devices=NCORES
    )
    xp_ap = nc.dram_tensor("xp", [BPC, CIN, NPAD], BF16, kind="ExternalInput").ap()
    w2_ap = nc.dram_tensor("w2", [128, TAPS, K], BF16, kind="ExternalInput").ap()
    off_ap = nc.dram_tensor("off", [K, 1], F32, kind="ExternalInput").ap()
    out_ap = nc.dram_tensor("out", [BPC, K, NOUT], F32, kind="ExternalOutput").ap()
    with tile.TileContext(nc) as tc:
        with ExitStack() as ctx:
            _conv_kernel(ctx, tc, out_ap, xp_ap, w2_ap, off_ap)
    nc.compile()
    _NC_CACHE = nc
    return nc


def _prep_inputs(x, weight, offset):
    """Host-side layout prep: pad x, transpose+duplicate weights, cast bf16."""
    x = np.ascontiguousarray(np.asarray(x, dtype=np.float32))
    weight = np.asarray(weight, dtype=np.float32)
    offset = np.asarray(offset, dtype=np.float32)

    xph = np.zeros((B, CIN, NPAD), dtype=ml_dtypes.bfloat16)
    xph[:, :, : HP * HP].reshape(B, CIN, HP, HP)[:, :, 1 : 1 + HW, 1 : 1 + HW] = x
    xph = np.ascontiguousarray(xph)

    wt = np.ascontiguousarray(weight.transpose(1, 2, 3, 0)).reshape(CIN, TAPS, K)
    w2 = np.ascontiguousarray(
        np.concatenate([wt, wt], axis=0).astype(ml_dtypes.bfloat16)
    )  # [128, 9, 128]
    off = np.ascontiguousarray(offset.reshape(K, 1))
    return xph, w2, off


def kernel(x, weight, offset):
    nc = _build_nc()
    xph, w2, off = _prep_inputs(x, weight, offset)
    in_maps = [
        {"xp": xph[i * BPC : (i + 1) * BPC], "w2": w2, "off": off}
        for i in range(NCORES)
    ]
    res = run_bass_kernel_spmd(nc, in_maps, list(range(NCORES))).results
    out = np.concatenate(
        [res[i]["out"].reshape(BPC, K, HW, HW) for i in range(NCORES)], axis=0
    )
    return out


# revision 7
# speedup vs baseline: 1.1043x; 1.0182x over previous
"""Trainium2 Bass kernel for nn_ConvLayer: 3x3 conv (stride 1, pad 1) + per-channel offset.

Problem: x[32,64,56,56] (*) w[128,64,3,3] + offset[128,1,1] -> out[32,128,56,56], fp32.

Strategy (8 NeuronCores, data-parallel over batch, 4 images/core):
  - Conv as 9 shifted matmuls (one per 3x3 tap) accumulated in PSUM.
  - CIN=64 -> each tap is a contract-64 matmul = half the 128x128 PE array.
    Two images are processed CONCURRENTLY via 64x128 row tiling: image A's
    channels live in SBUF partitions 0-63 (PE tile (0,0)), image B's in
    partitions 64-127 (PE tile (64,0)). Each accumulates into its own PSUM
    bank; each 64-row tile streams ~1 col/cycle, so the pair reaches full
    PE-array throughput (~196ns per 464-col matmul pair slot).
  - x and weights are cast to bf16 on the host (PSUM accumulation stays fp32;
    rel err ~2.5e-3 vs the 2e-2 gate). This halves input HBM traffic, shrinks
    the pipeline-fill head, and speeds weight loads.
  - Host pre-pads x to a 58x58 grid (zeros on borders) so every tap is a
    contiguous shifted window; host pre-transposes the weight to [cin,tap,k]
    (lhsT layout) and duplicates it into both partition halves.
  - The head races the ~200ns/tap matmul cadence: tap0-2 weights + the first
    x chunk ride the front of the two HWDGE rings, the rest of the weights
    and per-chunk x slices stream in just ahead of their consumers.
  - Per-chunk matmul groups (1 chunk = 8 output rows = one PSUM bank per
    image) keep the gating granularity small and the PSUM pipeline deep.
  - A ladder of decreasing-size warmup matmuls on scratch keeps TensorE
    continuously busy from the preamble barrier until real data lands, so
    the HAM clock gate (1.2 -> 2.4 GHz) opens as early as physics allows.
  - PSUM->SBUF eviction (ScalarE image A, VectorE image B) compacts the
    padded 58-wide rows to dense 56 and fuses the offset add; stores stream
    out per 1-2 chunks, image A on the Scalar ring, image B on Sync. The
    final store is split in row-halves to shorten the last-completion tail.
"""

import numpy as np
from contextlib import ExitStack

import ml_dtypes

import concourse.bass as bass
import concourse.tile as tile
from concourse import bacc, mybir
from concourse.bass_utils import run_bass_kernel_spmd

# Problem constants (hardcoded per contract).
B, CIN, HW, K = 32, 64, 56, 128
NCORES = 8
BPC = B // NCORES          # images per core
HP = HW + 2                # padded row width: 58
NPAD = HP * HP + 4         # padded image + slack for tap reads: 3368
NOUT = HW * HW             # 3136
ROWS_PER_CHUNK = 8
CHUNK = ROWS_PER_CHUNK * HP     # 464 <= 512 (one PSUM bank, fp32)
DCHUNK = ROWS_PER_CHUNK * HW    # 448 dense output cols per chunk
NCHUNKS = HW // ROWS_PER_CHUNK  # 7
TAPS = 9
F32 = mybir.dt.float32
BF16 = mybir.dt.bfloat16

# x-load slices, one per chunk: slice c covers every tap read of chunk c not
# already covered by earlier slices (chunk c reads cols [464c, 464c+582)).
XBOUNDS = [0, 584] + [464 * c + 120 for c in range(2, NCHUNKS)] + [NPAD]
# Output store ranges: (after_chunk, lo, hi) in dense cols.
STORES = [(0, 0, 448), (2, 448, 1344), (4, 1344, 2240), (5, 2240, 2688),
          (6, 2688, 3136)]

_NC_CACHE = None


def _conv_kernel(ctx: ExitStack, tc: "tile.TileContext", out_ap, xp_ap, w2_ap, off_ap):
    nc = tc.nc
    singles = ctx.enter_context(tc.tile_pool(name="singles", bufs=1))
    xpool = ctx.enter_context(tc.tile_pool(name="xpool", bufs=2))
    opool = ctx.enter_context(tc.tile_pool(name="opool", bufs=2))
    psum = ctx.enter_context(tc.tile_pool(name="psum", bufs=8, space="PSUM"))

    # Memset the warmup scratch first: it is SBUF-local and gates the first
    # warmup matmul, so it must not sit behind any DMA in program order.
    scratch = singles.tile([128, 512], BF16)
    nc.vector.memset(scratch[:], 0.0)

    # Head DMA order is critical: the first real matmul is gated by tap-0
    # weights + x slice 0, so those ride the front of their rings; later
    # weight taps and x chunks are ordered to complete just ahead of the
    # matmul that consumes them.
    #   Sync ring:   w taps0-2 -> x slice1 -> x slice3 -> x slice5 (+ B stores)
    #   Scalar ring: x slice0 -> w taps3-5 -> w taps6-8 -> off -> x2/x4/x6
    w_sb = singles.tile([128, TAPS, K], BF16)
    off_sb = singles.tile([128, 1], F32)
    x_ts = []
    x_srcs = []
    for pair in range(BPC // 2):
        x_ts.append(xpool.tile([128, NPAD], BF16, tag="x", name=f"x_{pair}"))
        x_srcs.append(xp_ap[2 * pair : 2 * pair + 2].rearrange("b c n -> (b c) n"))

    def load_x_slice(pair, s, eng):
        eng.dma_start(
            x_ts[pair][:, XBOUNDS[s] : XBOUNDS[s + 1]],
            x_srcs[pair][:, XBOUNDS[s] : XBOUNDS[s + 1]],
        )

    nc.sync.dma_start(w_sb[:, 0:3], w2_ap[:, 0:3])
    load_x_slice(0, 0, nc.scalar)
    nc.scalar.dma_start(w_sb[:, 3:6], w2_ap[:, 3:6])
    nc.scalar.dma_start(w_sb[:, 6:9], w2_ap[:, 6:9])
    nc.scalar.dma_start(off_sb[:], off_ap[:])

    # PE warmup: cheap bf16 matmuls on scratch keep TensorE busy during the
    # input-DMA head so the HAM clock gate opens early. The HAM busy-window
    # resets on ANY idle gap, so the warmups must bridge all the way to the
    # (jittery) first-input-landing time; sizes decrease so real matmuls
    # slot in with at most ~128 cycles of queue delay.
    ps_warm = psum.tile([128, 512], F32, tag="ps", name="ps_warm")
    for wn in (512, 512, 512, 512, 512, 256, 256, 256, 128, 128, 128, 128):
        nc.tensor.matmul(
            ps_warm[:, :wn], lhsT=scratch[0:64, 0:128], rhs=scratch[0:64, :wn],
            start=True, stop=True,
        )

    for pair in range(BPC // 2):
        b0 = 2 * pair
        # Both images of the pair side by side: [2, CIN, NPAD] -> [128, NPAD],
        # loaded as 7 per-chunk column slices alternating between the rings.
        x_t = x_ts[pair]
        for s in range(NCHUNKS):
            if pair == 0 and s == 0:
                continue  # already dispatched at the head
            eng = nc.sync if s % 2 == 1 else nc.scalar
            load_x_slice(pair, s, eng)
        o_sb = [
            opool.tile([128, NOUT], F32, tag="oA", name=f"oA_{pair}"),
            opool.tile([128, NOUT], F32, tag="oB", name=f"oB_{pair}"),
        ]
        stores = {c: (lo, hi) for c, lo, hi in STORES}

        for c in range(NCHUNKS):
            ps = [
                psum.tile([128, CHUNK], F32, tag="ps", name=f"ps_{pair}_{h}_{c}")
                for h in (0, 1)
            ]
            for t in range(TAPS):
                kh, kw = divmod(t, 3)
                o = kh * HP + kw + CHUNK * c
                st, sp = (t == 0), (t == TAPS - 1)
                for half in (0, 1):
                    lo, hi = 64 * half, 64 * half + 64
                    nc.tensor.matmul(
                        ps[half][:],
                        lhsT=w_sb[lo:hi, t, :],
                        rhs=x_t[lo:hi, o : o + CHUNK],
                        start=st,
                        stop=sp,
                    )
            # Evict: compact 58-wide padded rows to 56-wide dense rows and add
            # the per-channel offset. Image A on ScalarE, image B on VectorE.
            # The very last chunk of the last pair is evicted and stored in
            # row-halves so the final (critical-path) store completion covers
            # half the data and starts earlier.
            last_tail = pair == BPC // 2 - 1 and c == NCHUNKS - 1
            halves = ((0, 4), (4, 8)) if last_tail else ((0, 8),)
            for r0, r1 in halves:
                pa = ps[0].rearrange("p (r x) -> p r x", x=HP)[:, r0:r1, 0:HW]
                oa = o_sb[0][:, c * DCHUNK : (c + 1) * DCHUNK].rearrange(
                    "p (r x) -> p r x", x=HW
                )[:, r0:r1]
                nc.scalar.add(oa, pa, off_sb)
                pb = ps[1].rearrange("p (r x) -> p r x", x=HP)[:, r0:r1, 0:HW]
                ob = o_sb[1][:, c * DCHUNK : (c + 1) * DCHUNK].rearrange(
                    "p (r x) -> p r x", x=HW
                )[:, r0:r1]
                nc.vector.tensor_scalar_add(ob, pb, off_sb)
                if last_tail:
                    lo = c * DCHUNK + r0 * HW
                    hi = c * DCHUNK + r1 * HW
                    nc.scalar.dma_start(out_ap[b0][:, lo:hi], o_sb[0][:, lo:hi])
                    nc.sync.dma_start(out_ap[b0 + 1][:, lo:hi], o_sb[1][:, lo:hi])
            if c in stores and not last_tail:
                # Stream completed output out immediately. Image A rides the
                # Scalar HWDGE ring, image B the Sync ring, so the two output
                # streams (and the input stream) drain in parallel.
                lo, hi = stores[c]
                nc.scalar.dma_start(out_ap[b0][:, lo:hi], o_sb[0][:, lo:hi])
                nc.sync.dma_start(out_ap[b0 + 1][:, lo:hi], o_sb[1][:, lo:hi])


def _build_nc():
    global _NC_CACHE
    if _NC_CACHE is not None:
        return _NC_CACHE
    nc = bacc.Bacc(
        "TRN2", target_bir_lowering=False, debug=False, num_devices=NCORES
    )
    xp_ap = nc.dram_tensor("xp", [BPC, CIN, NPAD], BF16, kind="ExternalInput").ap()
    w2_ap = nc.dram_tensor("w2", [128, TAPS, K], BF16, kind="ExternalInput").ap()
    off_ap = nc.dram_tensor("off", [K, 1], F32, kind="ExternalInput").ap()
    out_ap = nc.dram_tensor("out", [BPC, K, NOUT], F32, kind="ExternalOutput").ap()
    with tile.TileContext(nc) as tc:
        with ExitStack() as ctx:
            _conv_kernel(ctx, tc, out_ap, xp_ap, w2_ap, off_ap)
    nc.compile()
    _NC_CACHE = nc
    return nc


def _prep_inputs(x, weight, offset):
    """Host-side layout prep: pad x, transpose+duplicate weights, cast bf16."""
    x = np.ascontiguousarray(np.asarray(x, dtype=np.float32))
    weight = np.asarray(weight, dtype=np.float32)
    offset = np.asarray(offset, dtype=np.float32)

    xph = np.zeros((B, CIN, NPAD), dtype=ml_dtypes.bfloat16)
    xph[:, :, : HP * HP].reshape(B, CIN, HP, HP)[:, :, 1 : 1 + HW, 1 : 1 + HW] = x
    xph = np.ascontiguousarray(xph)

    wt = np.ascontiguousarray(weight.transpose(1, 2, 3, 0)).reshape(CIN, TAPS, K)
    w2 = np.ascontiguousarray(
        np.concatenate([wt, wt], axis=0).astype(ml_dtypes.bfloat16)
    )  # [128, 9, 128]
    off = np.ascontiguousarray(offset.reshape(K, 1))
    return xph, w2, off


def kernel(x, weight, offset):
    nc = _build_nc()
    xph, w2, off = _prep_inputs(x, weight, offset)
    in_maps = [
        {"xp": xph[i * BPC : (i + 1) * BPC], "w2": w2, "off": off}
        for i in range(NCORES)
    ]
    res = run_bass_kernel_spmd(nc, in_maps, list(range(NCORES))).results
    out = np.concatenate(
        [res[i]["out"].reshape(BPC, K, HW, HW) for i in range(NCORES)], axis=0
    )
    return out


# revision 8
# speedup vs baseline: 1.1049x; 1.0005x over previous
"""Trainium2 Bass kernel for nn_ConvLayer: 3x3 conv (stride 1, pad 1) + per-channel offset.

Problem: x[32,64,56,56] (*) w[128,64,3,3] + offset[128,1,1] -> out[32,128,56,56], fp32.

Strategy (8 NeuronCores, data-parallel over batch, 4 images/core):
  - Conv as 9 shifted matmuls (one per 3x3 tap) accumulated in PSUM.
  - CIN=64 -> each tap is a contract-64 matmul = half the 128x128 PE array.
    Two images are processed CONCURRENTLY via 64x128 row tiling: image A's
    channels live in SBUF partitions 0-63 (PE tile (0,0)), image B's in
    partitions 64-127 (PE tile (64,0)). Each accumulates into its own PSUM
    bank; each 64-row tile streams ~1 col/cycle, so the pair reaches full
    PE-array throughput (~190ns per 456-col matmul pair slot).
  - x and weights are cast to bf16 on the host (PSUM accumulation stays fp32;
    rel err ~2.5e-3 vs the 2e-2 gate). This halves input HBM traffic, shrinks
    the pipeline-fill head, and speeds weight loads.
  - Host pre-pads x to a 58-row, 57-pitch packed grid (zeros on borders;
    adjacent rows share one pad column) so every tap is a
    contiguous shifted window; host pre-transposes the weight to [cin,tap,k]
    (lhsT layout) and duplicates it into both partition halves.
  - The head races the ~200ns/tap matmul cadence: tap0-2 weights + the first
    x chunk ride the front of the two HWDGE rings, the rest of the weights
    and per-chunk x slices stream in just ahead of their consumers.
  - Per-chunk matmul groups (1 chunk = 8 output rows = one PSUM bank per
    image) keep the gating granularity small and the PSUM pipeline deep.
  - A ladder of decreasing-size warmup matmuls on scratch keeps TensorE
    continuously busy from the preamble barrier until real data lands, so
    the HAM clock gate (1.2 -> 2.4 GHz) opens as early as physics allows.
  - PSUM->SBUF eviction (ScalarE image A, VectorE image B) compacts the
    packed 57-pitch rows to dense 56 and fuses the offset add; stores stream
    out per 1-2 chunks, image A on the Scalar ring, image B on Sync. The
    final store is split in row-halves to shorten the last-completion tail.
"""

import numpy as np
from contextlib import ExitStack

import ml_dtypes

import concourse.bass as bass
import concourse.tile as tile
from concourse import bacc, mybir
from concourse.bass_utils import run_bass_kernel_spmd

# Problem constants (hardcoded per contract).
B, CIN, HW, K = 32, 64, 56, 128
NCORES = 8
BPC = B // NCORES          # images per core
HP = HW + 1                # packed padded row pitch: 57 (rows share one pad col:
                           # col 57r is row r's left pad AND row r-1's right pad)
NPAD = HP * 58 + 6         # 58 padded rows + slack for tap reads: 3312
NOUT = HW * HW             # 3136
ROWS_PER_CHUNK = 8
CHUNK = ROWS_PER_CHUNK * HP     # 456 <= 512 (one PSUM bank, fp32)
DCHUNK = ROWS_PER_CHUNK * HW    # 448 dense output cols per chunk
NCHUNKS = HW // ROWS_PER_CHUNK  # 7
TAPS = 9
F32 = mybir.dt.float32
BF16 = mybir.dt.bfloat16

# x-load slices, one per chunk: slice c covers every tap read of chunk c not
# already covered by earlier slices (chunk c reads cols [456c, 456c+570)).
XBOUNDS = [0, 576] + [CHUNK * c + 120 for c in range(2, NCHUNKS)] + [NPAD]
# Output store ranges: (after_chunk, lo, hi) in dense cols.
STORES = [(0, 0, 448), (2, 448, 1344), (4, 1344, 2240), (5, 2240, 2688),
          (6, 2688, 3136)]

_NC_CACHE = None


def _conv_kernel(ctx: ExitStack, tc: "tile.TileContext", out_ap, xp_ap, w2_ap, off_ap):
    nc = tc.nc
    singles = ctx.enter_context(tc.tile_pool(name="singles", bufs=1))
    xpool = ctx.enter_context(tc.tile_pool(name="xpool", bufs=2))
    opool = ctx.enter_context(tc.tile_pool(name="opool", bufs=2))
    psum = ctx.enter_context(tc.tile_pool(name="psum", bufs=8, space="PSUM"))

    # Memset the warmup scratch first: it is SBUF-local and gates the first
    # warmup matmul, so it must not sit behind any DMA in program order.
    scratch = singles.tile([128, 512], BF16)
    nc.vector.memset(scratch[:], 0.0)

    # Head DMA order is critical: the first real matmul is gated by tap-0
    # weights + x slice 0, so those ride the front of their rings; later
    # weight taps and x chunks are ordered to complete just ahead of the
    # matmul that consumes them.
    #   Sync ring:   w taps0-2 -> x slice1 -> x slice3 -> x slice5 (+ B stores)
    #   Scalar ring: x slice0 -> w taps3-5 -> w taps6-8 -> off -> x2/x4/x6
    w_sb = singles.tile([128, TAPS, K], BF16)
    off_sb = singles.tile([128, 1], F32)
    x_ts = []
    x_srcs = []
    for pair in range(BPC // 2):
        x_ts.append(xpool.tile([128, NPAD], BF16, tag="x", name=f"x_{pair}"))
        x_srcs.append(xp_ap[2 * pair : 2 * pair + 2].rearrange("b c n -> (b c) n"))

    def load_x_slice(pair, s, eng):
        eng.dma_start(
            x_ts[pair][:, XBOUNDS[s] : XBOUNDS[s + 1]],
            x_srcs[pair][:, XBOUNDS[s] : XBOUNDS[s + 1]],
        )

    nc.sync.dma_start(w_sb[:, 0:3], w2_ap[:, 0:3])
    load_x_slice(0, 0, nc.scalar)
    nc.scalar.dma_start(w_sb[:, 3:6], w2_ap[:, 3:6])
    nc.scalar.dma_start(w_sb[:, 6:9], w2_ap[:, 6:9])
    nc.scalar.dma_start(off_sb[:], off_ap[:])

    # PE warmup: cheap bf16 matmuls on scratch keep TensorE busy during the
    # input-DMA head so the HAM clock gate opens early. The HAM busy-window
    # resets on ANY idle gap, so the warmups must bridge all the way to the
    # (jittery) first-input-landing time; sizes decrease so real matmuls
    # slot in with at most ~128 cycles of queue delay.
    ps_warm = psum.tile([128, 512], F32, tag="ps", name="ps_warm")
    for wn in (512, 512, 512, 512, 512, 256, 256, 256, 128, 128, 128, 128):
        nc.tensor.matmul(
            ps_warm[:, :wn], lhsT=scratch[0:64, 0:128], rhs=scratch[0:64, :wn],
            start=True, stop=True,
        )

    for pair in range(BPC // 2):
        b0 = 2 * pair
        # Both images of the pair side by side: [2, CIN, NPAD] -> [128, NPAD],
        # loaded as 7 per-chunk column slices alternating between the rings.
        x_t = x_ts[pair]
        for s in range(NCHUNKS):
            if pair == 0 and s == 0:
                continue  # already dispatched at the head
            eng = nc.sync if s % 2 == 1 else nc.scalar
            load_x_slice(pair, s, eng)
        o_sb = [
            opool.tile([128, NOUT], F32, tag="oA", name=f"oA_{pair}"),
            opool.tile([128, NOUT], F32, tag="oB", name=f"oB_{pair}"),
        ]
        stores = {c: (lo, hi) for c, lo, hi in STORES}

        for c in range(NCHUNKS):
            ps = [
                psum.tile([128, CHUNK], F32, tag="ps", name=f"ps_{pair}_{h}_{c}")
                for h in (0, 1)
            ]
            for t in range(TAPS):
                kh, kw = divmod(t, 3)
                o = kh * HP + kw + CHUNK * c
                st, sp = (t == 0), (t == TAPS - 1)
                for half in (0, 1):
                    lo, hi = 64 * half, 64 * half + 64
                    nc.tensor.matmul(
                        ps[half][:],
                        lhsT=w_sb[lo:hi, t, :],
                        rhs=x_t[lo:hi, o : o + CHUNK],
                        start=st,
                        stop=sp,
                    )
            # Evict: compact 57-pitch packed rows to 56-wide dense rows and add
            # the per-channel offset. Image A on ScalarE, image B on VectorE.
            # The very last chunk of the last pair is evicted and stored in
            # row-halves so the final (critical-path) store completion covers
            # half the data and starts earlier.
            last_tail = pair == BPC // 2 - 1 and c == NCHUNKS - 1
            halves = ((0, 4), (4, 8)) if last_tail else ((0, 8),)
            for r0, r1 in halves:
                pa = ps[0].rearrange("p (r x) -> p r x", x=HP)[:, r0:r1, 0:HW]
                oa = o_sb[0][:, c * DCHUNK : (c + 1) * DCHUNK].rearrange(
                    "p (r x) -> p r x", x=HW
                )[:, r0:r1]
                nc.scalar.add(oa, pa, off_sb)
                pb = ps[1].rearrange("p (r x) -> p r x", x=HP)[:, r0:r1, 0:HW]
                ob = o_sb[1][:, c * DCHUNK : (c + 1) * DCHUNK].rearrange(
                    "p (r x) -> p r x", x=HW
                )[:, r0:r1]
                nc.vector.tensor_scalar_add(ob, pb, off_sb)
                if last_tail:
                    lo = c * DCHUNK + r0 * HW
                    hi = c * DCHUNK + r1 * HW
                    nc.scalar.dma_start(out_ap[b0][:, lo:hi], o_sb[0][:, lo:hi])
                    nc.sync.dma_start(out_ap[b0 + 1][:, lo:hi], o_sb[1][:, lo:hi])
            if c in stores and not last_tail:
                # Stream completed output out immediately. Image A rides the
                # Scalar HWDGE ring, image B the Sync ring, so the two output
                # streams (and the input stream) drain in parallel.
                lo, hi = stores[c]
                nc.scalar.dma_start(out_ap[b0][:, lo:hi], o_sb[0][:, lo:hi])
                nc.sync.dma_start(out_ap[b0 + 1][:, lo:hi], o_sb[1][:, lo:hi])


def _build_nc():
    global _NC_CACHE
    if _NC_CACHE is not None:
        return _NC_CACHE
    nc = bacc.Bacc(
        "TRN2", target_bir_lowering=False, debug=False, num_devices=NCORES
    )
    xp_ap = nc.dram_tensor("xp", [BPC, CIN, NPAD], BF16, kind="ExternalInput").ap()
    w2_ap = nc.dram_tensor("w2", [128, TAPS, K], BF16, kind="ExternalInput").ap()
    off_ap = nc.dram_tensor("off", [K, 1], F32, kind="ExternalInput").ap()
    out_ap = nc.dram_tensor("out", [BPC, K, NOUT], F32, kind="ExternalOutput").ap()
    with tile.TileContext(nc) as tc:
        with ExitStack() as ctx:
            _conv_kernel(ctx, tc, out_ap, xp_ap, w2_ap, off_ap)
    nc.compile()
    _NC_CACHE = nc
    return nc


def _prep_inputs(x, weight, offset):
    """Host-side layout prep: pad x, transpose+duplicate weights, cast bf16."""
    x = np.ascontiguousarray(np.asarray(x, dtype=np.float32))
    weight = np.asarray(weight, dtype=np.float32)
    offset = np.asarray(offset, dtype=np.float32)

    xph = np.zeros((B, CIN, NPAD), dtype=ml_dtypes.bfloat16)
    xph[:, :, : HP * 58].reshape(B, CIN, 58, HP)[:, :, 1 : 1 + HW, 1 : 1 + HW] = x
    xph = np.ascontiguousarray(xph)

    wt = np.ascontiguousarray(weight.transpose(1, 2, 3, 0)).reshape(CIN, TAPS, K)
    w2 = np.ascontiguousarray(
        np.concatenate([wt, wt], axis=0).astype(ml_dtypes.bfloat16)
    )  # [128, 9, 128]
    off = np.ascontiguousarray(offset.reshape(K, 1))
    return xph, w2, off


def kernel(x, weight, offset):
    nc = _build_nc()
    xph, w2, off = _prep_inputs(x, weight, offset)
    in_maps = [
        {"xp": xph[i * BPC : (i + 1) * BPC], "w2": w2, "off": off}
        for i in range(NCORES)
    ]
    res = run_bass_kernel_spmd(nc, in_maps, list(range(NCORES))).results
    out = np.concatenate(
        [res[i]["out"].reshape(BPC, K, HW, HW) for i in range(NCORES)], axis=0
    )
    return out
